# revision 31
# baseline (speedup 1.0000x reference)
"""Trainium2 Bass kernel for nn_EquiAttentionBlock (N=1024, H=128, 8 NeuronCores).

Math (per row-block of 128 rows i per core, all j):
  u   = x @ We1.T                         [N, H]
  e[i,j,:] = u[i] - u[j] + be1            (edge-MLP hidden, pre-relu)
  cw_u[i,j] = relu(e[i,j,:]) . w2c        where w2c = (Wc @ We2).T  [H]
  coord_w[i,j] = cw_u[i,j] + b2c          where b2c = Wc @ be2 + bc (scalar)
  logits = (q @ k.T) * 1/sqrt(H); ae = exp(logits - max); se = sum_j ae
  out_h[i] = h[i] + bv + (ae @ v0)[i] / se_i          (v0 = h @ Wv.T, no bias)
  delta_x[i] = 1/se_i * ( x_i * (sum_j ae*cw_u + b2c*se_i)
                          - (sum_j ae*cw_u*x_j + b2c*(ae @ x)_i) )
  out_x[i] = x[i] + delta_x[i]

Layout: d (=H=128) on SBUF partitions for the relu stream; per-i relu tile
[128 d, 1024 j] is produced by ONE fused op (ACT activation w/ per-partition
bias, or DVE tensor_scalar) and contracted over d by PE matmuls
(column-tiled: 4 concurrent M=32 matmuls via a sliding-window weight tile).
Inputs are packed into 3 DMA transfers; both outputs into 1.
"""

import numpy as np
import ml_dtypes

N, H, NCORES = 1024, 128, 8
BLK = N // NCORES  # 128
P = 128
SCALE = float(1.0 / np.sqrt(np.float32(H)))

CONFIG = dict(
    act_groups=(3,),     # relu col-groups produced on ScalarE
    act_extra_q=0,       # ACT also takes group 2 for q < this
    act_bf16=False,      # ACT relu reads the bf16 negated copy (2x probe)
    bench_half_mm=False,  # bench-only: half the relu matmuls (wrong numerics)
    bench_one_relu=False,  # bench-only: one relu tile per quad (wrong numerics)
    bench_dve_1op=False,  # bench-only: single-op DVE relu (wrong numerics)
    fused_transpose=True,  # single 3D dma transpose
    rp_bufs=16,          # relu tile pool depth
    gp_q2=0,             # GPSIMD takes j==2 tiles with q >= 32-gp_q2
    no_memset=True,      # rely on start=True at q==0 instead of PSUM memset
)

# packB (bf16, [128, PB_COLS]) column offsets
PB_HT = 0             # hT [H, N]
PB_HTB = 1024         # hTb [H, BLK]
PB_WQT = 1152         # Wq.T [H, H]
PB_WKT = 1280
PB_WVT = 1408
PB_ZW = 1536          # Zw [H, 63] (w2c at col 31, zeros around)
PB_XO = 1600          # xo rearranged [P, 8, 3] -> cols 3n+c
PB_XV = 1624          # x bf16 rearranged [P, 8, 2] -> cols 2n+c
PB_COLS = 1640

# packF (f32, [128, PF_COLS]) column offsets
PF_HB = 0             # h block + bv [BLK, H]
PF_XB = 128           # x block [BLK, 2]
PF_BQS = 130          # SCALE * bq [H,1]
PF_BK = 131
PF_BE1 = 132
PF_BE1N = 133
PF_B2C = 134
PF_COLS = 135

# packX (f32, [2, PX_COLS]) column offsets
PX_XT = 0             # x.T [2, N]
PX_XTB = 1024         # x_block.T [2, BLK]
PX_WE1T = 1152        # We1.T [2, H]
PX_COLS = 1280

_CACHE = {}


def _build_program(loop_reps=1, **overrides):
    cfg = dict(CONFIG)
    cfg.update(overrides)

    import concourse.tile as tile
    from concourse import bacc, mybir
    from concourse.bass import ts

    f32 = mybir.dt.float32
    bf16 = mybir.dt.bfloat16
    AF = mybir.ActivationFunctionType
    OP = mybir.AluOpType
    AX = mybir.AxisListType

    nc = bacc.Bacc("TRN2")

    packB = nc.dram_tensor("packB", [H, PB_COLS], bf16, kind="ExternalInput")
    packF = nc.dram_tensor("packF", [BLK, PF_COLS], f32, kind="ExternalInput")
    packX = nc.dram_tensor("packX", [2, PX_COLS], f32, kind="ExternalInput")
    out = nc.dram_tensor("out", [BLK, H + 2], f32, kind="ExternalOutput")

    with tile.TileContext(nc) as tc:
        with (
            tc.tile_pool(name="dp", bufs=1) as dp,
            tc.tile_pool(name="rp", bufs=cfg["rp_bufs"]) as rp,
            tc.tile_pool(name="ps_big", bufs=1, space="PSUM") as ps_big,
            tc.tile_pool(name="ps_cw", bufs=1, space="PSUM") as ps_cw,
            tc.tile_pool(name="ps_sm", bufs=2, space="PSUM") as ps_sm,
            tc.tile_pool(name="ps_acc", bufs=1, space="PSUM") as ps_acc,
            tc.tile_pool(name="ps_dx", bufs=1, space="PSUM") as ps_dx,
        ):
            def body():
                # ---------------- packed SBUF loads ----------------
                pB = dp.tile([H, PB_COLS], bf16, tag="pB")
                pF = dp.tile([BLK, PF_COLS], f32, tag="pF")
                pX = dp.tile([2, PX_COLS], f32, tag="pX")
                nc.sync.dma_start(pB[:], packB[:])
                nc.sync.dma_start(pF[:], packF[:])
                nc.sync.dma_start(pX[:], packX[:])

                hT_s = pB[:, PB_HT : PB_HT + N]
                hTb_s = pB[:, PB_HTB : PB_HTB + BLK]
                WqT_s = pB[:, PB_WQT : PB_WQT + H]
                WkT_s = pB[:, PB_WKT : PB_WKT + H]
                WvT_s = pB[:, PB_WVT : PB_WVT + H]
                Z_s = pB[:, PB_ZW : PB_ZW + 63]
                xo_s = pB[:, PB_XO : PB_XO + 24].rearrange("p (n c) -> p n c", n=8)
                xv_s = pB[:, PB_XV : PB_XV + 16].rearrange("p (n c) -> p n c", n=8)
                hb_s = pF[:, PF_HB : PF_HB + H]
                xb_s = pF[:, PF_XB : PF_XB + 2]
                bqs_s = pF[:, PF_BQS : PF_BQS + 1]
                bk_s = pF[:, PF_BK : PF_BK + 1]
                be1_s = pF[:, PF_BE1 : PF_BE1 + 1]
                be1n_s = pF[:, PF_BE1N : PF_BE1N + 1]
                b2c_s = pF[:, PF_B2C : PF_B2C + 1]
                xT_s = pX[:, PX_XT : PX_XT + N]
                xTb_s = pX[:, PX_XTB : PX_XTB + BLK]
                We1T_s = pX[:, PX_WE1T : PX_WE1T + H]

                # [v0 | x] bf16, chunked by j (v0 filled below from psum)
                vx_s = dp.tile([P, 8, H + 2], bf16, tag="vx_s")
                nc.vector.tensor_copy(vx_s[:, :, H : H + 2], xv_s[:])

                # ---------------- u^T and per-i biases ----------------
                uT_ps = ps_big.tile([H, N], f32, tag="big")
                for half in range(2):
                    nc.tensor.matmul(
                        out=uT_ps[:, ts(half, 512)],
                        lhsT=We1T_s,
                        rhs=xT_s[:, ts(half, 512)],
                        start=True,
                        stop=True,
                    )
                uT_s = dp.tile([H, N], f32, tag="uT_s")
                uTn_b = dp.tile([H, N], bf16, tag="uTn_b")
                nc.scalar.copy(uT_s[:], uT_ps[:])
                nc.scalar.mul(uTn_b[:], uT_ps[:], -1.0)

                uTb_ps = ps_sm.tile([H, BLK], f32, tag="sm")
                nc.tensor.matmul(
                    out=uTb_ps[:], lhsT=We1T_s, rhs=xTb_s, start=True, stop=True
                )
                B_s = dp.tile([H, BLK], f32, tag="B_s")
                Bn_s = dp.tile([H, BLK], f32, tag="Bn_s")
                nc.scalar.activation(
                    out=B_s[:], in_=uTb_ps[:], func=AF.Identity,
                    bias=be1_s, scale=1.0,
                )
                nc.scalar.activation(
                    out=Bn_s[:], in_=uTb_ps[:], func=AF.Identity,
                    bias=be1n_s, scale=-1.0,
                )

                # ---------------- q, k, v ----------------
                qT_ps = ps_sm.tile([H, BLK], f32, tag="sm")
                nc.tensor.matmul(
                    out=qT_ps[:], lhsT=WqT_s, rhs=hTb_s, start=True, stop=True
                )
                qT_s = dp.tile([H, BLK], bf16, tag="qT_s")
                nc.scalar.activation(
                    out=qT_s[:], in_=qT_ps[:], func=AF.Identity,
                    bias=bqs_s, scale=SCALE,
                )

                kT_ps = ps_big.tile([H, N], f32, tag="big")
                for half in range(2):
                    nc.tensor.matmul(
                        out=kT_ps[:, ts(half, 512)],
                        lhsT=WkT_s,
                        rhs=hT_s[:, ts(half, 512)],
                        start=True,
                        stop=True,
                    )
                kT_s = dp.tile([H, N], bf16, tag="kT_s")
                nc.scalar.activation(
                    out=kT_s[:], in_=kT_ps[:], func=AF.Identity,
                    bias=bk_s, scale=1.0,
                )

                for n in range(8):
                    v_ps = ps_sm.tile([P, H], f32, tag="sm")
                    nc.tensor.matmul(
                        out=v_ps[:], lhsT=hT_s[:, ts(n, P)], rhs=WvT_s,
                        start=True, stop=True,
                    )
                    nc.scalar.copy(vx_s[:, n, 0:H], v_ps[:])

                # ---------------- attention logits + softmax ----------------
                lg_ps = ps_big.tile([BLK, N], f32, tag="big")
                for half in range(2):
                    nc.tensor.matmul(
                        out=lg_ps[:, ts(half, 512)],
                        lhsT=qT_s[:],
                        rhs=kT_s[:, ts(half, 512)],
                        start=True,
                        stop=True,
                    )
                mx_s = dp.tile([BLK, 1], f32, tag="mx_s")
                nc.vector.tensor_reduce(
                    out=mx_s[:], in_=lg_ps[:], axis=AX.X, op=OP.max, negate=True
                )
                attn_b = dp.tile([BLK, N], bf16, tag="attn_b")
                se_s = dp.tile([BLK, 1], f32, tag="se_s")
                nc.scalar.activation(
                    out=attn_b[:],
                    in_=lg_ps[:],
                    func=AF.Exp,
                    bias=mx_s[:, 0:1],
                    scale=1.0,
                    accum_out=se_s[:, 0:1],
                )
                rs_s = dp.tile([BLK, 1], f32, tag="rs_s")
                nc.vector.reciprocal(rs_s[:], se_s[:])

                # transposed attention chunks (j on partitions) via DMA xbar
                aT_s = dp.tile([P, 8, BLK], bf16, tag="aT_s")
                if cfg["fused_transpose"]:
                    nc.sync.dma_start_transpose(aT_s[:], attn_b[:])
                else:
                    for n in range(8):
                        nc.sync.dma_start_transpose(
                            aT_s[:, n, :], attn_b[:, ts(n, P)]
                        )

                # ---------------- main relu/contract loop ----------------
                cw_ps = ps_cw.tile([BLK, N], f32, tag="cw")
                if not cfg["no_memset"]:
                    # Pre-zero; every matmul then either overwrites
                    # (has_written clear) or accumulates onto zeros — correct
                    # in both HW semantics and CoreSim's pending-zero model.
                    nc.vector.memset(cw_ps[:], 0.0)

                for q in range(32):
                    rts = []
                    for j in range(1 if cfg["bench_one_relu"] else 4):
                        p = 32 * j + q
                        rt = rp.tile([H, N], bf16, tag="relu")
                        on_act = j in cfg["act_groups"] or (
                            j == 2 and q < cfg["act_extra_q"]
                        )
                        on_gp = (not on_act) and (
                            j == 2 and q >= 32 - cfg["gp_q2"]
                        )
                        if on_act:
                            if cfg["act_bf16"]:
                                nc.scalar.activation(
                                    out=rt[:],
                                    in_=uTn_b[:],
                                    func=AF.Relu,
                                    bias=B_s[:, p : p + 1],
                                    scale=1.0,
                                )
                            else:
                                nc.scalar.activation(
                                    out=rt[:],
                                    in_=uT_s[:],
                                    func=AF.Relu,
                                    bias=B_s[:, p : p + 1],
                                    scale=-1.0,
                                )
                        elif cfg["bench_dve_1op"]:
                            eng = nc.gpsimd if on_gp else nc.vector
                            eng.tensor_scalar(
                                out=rt[:],
                                in0=uTn_b[:],
                                scalar1=Bn_s[:, p : p + 1],
                                scalar2=None,
                                op0=OP.max,
                            )
                        else:
                            eng = nc.gpsimd if on_gp else nc.vector
                            eng.tensor_scalar(
                                out=rt[:],
                                in0=uTn_b[:],
                                scalar1=Bn_s[:, p : p + 1],
                                scalar2=0.0,
                                op0=OP.subtract,
                                op1=OP.max,
                            )
                        rts.append(rt)
                    for j in range(4):
                        for half in (0,) if cfg["bench_half_mm"] else (0, 1):
                            nc.tensor.matmul(
                                out=cw_ps[32 * j : 32 * j + 32, ts(half, 512)],
                                lhsT=Z_s[:, 31 - q : 63 - q],
                                rhs=rts[j if not cfg["bench_one_relu"] else 0][
                                    :, ts(half, 512)
                                ],
                                start=(cfg["no_memset"] and q == 0),
                                stop=(q == 31 and j == 3),
                                tile_position=(0, 32 * j),
                                skip_group_check=True,
                            )

                # ---------------- delta_x path ----------------
                tu_b = dp.tile([BLK, N], bf16, tag="tu_b")
                nc.vector.tensor_tensor(
                    out=tu_b[:], in0=attn_b[:], in1=cw_ps[:], op=OP.mult
                )
                tT_s = dp.tile([P, 8, BLK], bf16, tag="tT_s")
                if cfg["fused_transpose"]:
                    nc.sync.dma_start_transpose(tT_s[:], tu_b[:])
                else:
                    for n in range(8):
                        nc.sync.dma_start_transpose(
                            tT_s[:, n, :], tu_b[:, ts(n, P)]
                        )

                agg_ps = ps_acc.tile([BLK, H + 2], f32, tag="agg")
                for n in range(8):
                    nc.tensor.matmul(
                        out=agg_ps[:],
                        lhsT=aT_s[:, n, :],
                        rhs=vx_s[:, n, :],
                        start=(n == 0),
                        stop=(n == 7),
                    )
                dx_ps = ps_dx.tile([BLK, 3], f32, tag="dx")
                for n in range(8):
                    nc.tensor.matmul(
                        out=dx_ps[:],
                        lhsT=tT_s[:, n, :],
                        rhs=xo_s[:, n, :],
                        start=(n == 0),
                        stop=(n == 7),
                    )

                # ---------------- final combines (one [BLK, H+2] tile) ------
                o_s = dp.tile([BLK, H + 2], f32, tag="o_s")
                nc.vector.tensor_scalar_mul(
                    o_s[:, 0:H], agg_ps[:, 0:H], rs_s[:, 0:1]
                )
                nc.vector.tensor_tensor(
                    out=o_s[:, 0:H], in0=o_s[:, 0:H], in1=hb_s, op=OP.add
                )

                # A1 = se*b2c + sum_t ; T1 = xb*A1
                A1 = dp.tile([BLK, 1], f32, tag="A1")
                nc.vector.tensor_tensor(
                    out=A1[:], in0=se_s[:], in1=b2c_s, op=OP.mult
                )
                nc.vector.tensor_tensor(
                    out=A1[:], in0=A1[:], in1=dx_ps[:, 2:3], op=OP.add
                )
                T1 = dp.tile([BLK, 2], f32, tag="T1")
                nc.vector.tensor_scalar_mul(T1[:], xb_s, A1[:, 0:1])
                # T2 = b2c*(ae@x) + sum_t_x
                T2 = dp.tile([BLK, 2], f32, tag="T2")
                nc.vector.tensor_scalar_mul(
                    T2[:], agg_ps[:, H : H + 2], b2c_s
                )
                nc.vector.tensor_tensor(
                    out=T2[:], in0=T2[:], in1=dx_ps[:, 0:2], op=OP.add
                )
                nc.vector.tensor_tensor(
                    out=o_s[:, H : H + 2], in0=T1[:], in1=T2[:], op=OP.subtract
                )
                nc.vector.tensor_scalar_mul(
                    o_s[:, H : H + 2], o_s[:, H : H + 2], rs_s[:, 0:1]
                )
                nc.vector.tensor_tensor(
                    out=o_s[:, H : H + 2], in0=o_s[:, H : H + 2], in1=xb_s,
                    op=OP.add,
                )
                nc.sync.dma_start(out[:], o_s[:])

            if loop_reps == 1:
                body()
            else:
                with tc.For_i(
                    0, loop_reps, 1,
                    hint_engines=(
                        nc.tensor.engine, nc.vector.engine, nc.scalar.engine
                    ),
                ):
                    body()

    nc.finalize()
    return nc


def get_program(loop_reps=1, **overrides):
    overrides = {
        k: tuple(v) if isinstance(v, list) else v for k, v in overrides.items()
    }
    key = ("nc", loop_reps, tuple(sorted(overrides.items())))
    if key not in _CACHE:
        _CACHE[key] = _build_program(loop_reps, **overrides)
    return _CACHE[key]


def make_in_maps(inputs):
    bf = ml_dtypes.bfloat16
    f32 = np.float32
    h = np.ascontiguousarray(np.asarray(inputs["h"], f32))
    x = np.ascontiguousarray(np.asarray(inputs["x"], f32))
    Wq = np.asarray(inputs["Wq"], f32)
    bq = np.asarray(inputs["bq"], f32)
    Wk = np.asarray(inputs["Wk"], f32)
    bk = np.asarray(inputs["bk"], f32)
    Wv = np.asarray(inputs["Wv"], f32)
    bv = np.asarray(inputs["bv"], f32)
    We1 = np.asarray(inputs["We1"], f32)
    be1 = np.asarray(inputs["be1"], f32)
    We2 = np.asarray(inputs["We2"], f32)
    be2 = np.asarray(inputs["be2"], f32)
    Wc = np.asarray(inputs["Wc"], f32)
    bc = np.asarray(inputs["bc"], f32)

    w2c = (Wc @ We2).reshape(H)                    # [H]
    b2c_val = float((Wc @ be2)[0] + bc[0])

    # packB (bf16)
    pb = np.zeros((H, PB_COLS), f32)
    pb[:, PB_HT : PB_HT + N] = h.T
    pb[:, PB_WQT : PB_WQT + H] = Wq.T
    pb[:, PB_WKT : PB_WKT + H] = Wk.T
    pb[:, PB_WVT : PB_WVT + H] = Wv.T
    pb[:, PB_ZW + 31] = w2c
    # xo: [x0, x1, 1] chunk-rearranged; xv: x chunk-rearranged
    x3 = np.concatenate([x, np.ones((N, 1), f32)], axis=1)  # [N, 3]
    pb[:, PB_XO : PB_XO + 24] = x3.reshape(8, P, 3).transpose(1, 0, 2).reshape(P, 24)
    pb[:, PB_XV : PB_XV + 16] = x.reshape(8, P, 2).transpose(1, 0, 2).reshape(P, 16)

    # packX (f32)
    px = np.zeros((2, PX_COLS), f32)
    px[:, PX_XT : PX_XT + N] = x.T
    px[:, PX_WE1T : PX_WE1T + H] = We1.T

    in_maps = []
    for c in range(NCORES):
        sl = slice(c * BLK, (c + 1) * BLK)
        pbc = pb.copy()
        pbc[:, PB_HTB : PB_HTB + BLK] = h[sl].T
        pf = np.zeros((BLK, PF_COLS), f32)
        pf[:, PF_HB : PF_HB + H] = h[sl] + bv[None, :]
        pf[:, PF_XB : PF_XB + 2] = x[sl]
        pf[:, PF_BQS] = SCALE * bq
        pf[:, PF_BK] = bk
        pf[:, PF_BE1] = be1
        pf[:, PF_BE1N] = -be1
        pf[:, PF_B2C] = b2c_val
        pxc = px.copy()
        pxc[:, PX_XTB : PX_XTB + BLK] = x[sl].T
        in_maps.append(
            {
                "packB": np.ascontiguousarray(pbc).astype(bf),
                "packF": np.ascontiguousarray(pf),
                "packX": np.ascontiguousarray(pxc),
            }
        )
    return in_maps


def kernel(**inputs):
    from concourse.bass_utils import run_bass_kernel_spmd

    nc = get_program()
    in_maps = make_in_maps(inputs)
    res = run_bass_kernel_spmd(nc, in_maps, core_ids=list(range(NCORES)))
    o = np.concatenate([r["out"] for r in res.results], axis=0)
    return (
        np.ascontiguousarray(o[:, 0:H]).astype(np.float32),
        np.ascontiguousarray(o[:, H : H + 2]).astype(np.float32),
    )


# revision 34
# speedup vs baseline: 1.4027x; 1.4027x over previous
"""Trainium2 Bass kernel for nn_EquiAttentionBlock (N=1024, H=128, 8 NeuronCores).

Math (per row-block of 128 rows i per core, all j):
  u   = x @ We1.T                         [N, H]
  e[i,j,:] = u[i] - u[j] + be1            (edge-MLP hidden, pre-relu)
  cw_u[i,j] = relu(e[i,j,:]) . w2c        where w2c = (Wc @ We2).T  [H]
  coord_w[i,j] = cw_u[i,j] + b2c          where b2c = Wc @ be2 + bc (scalar)
  logits = (q @ k.T) * 1/sqrt(H); ae = exp(logits - max); se = sum_j ae
  out_h[i] = h[i] + bv + (ae @ v0)[i] / se_i          (v0 = h @ Wv.T, no bias)
  delta_x[i] = 1/se_i * ( x_i * (sum_j ae*cw_u + b2c*se_i)
                          - (sum_j ae*cw_u*x_j + b2c*(ae @ x)_i) )
  out_x[i] = x[i] + delta_x[i]

Layout: d (=H=128) on SBUF partitions for the relu stream; per-i relu tile
[128 d, 1024 j] is produced by ONE fused op (ACT activation w/ per-partition
bias, or DVE tensor_scalar) and contracted over d by PE matmuls
(column-tiled: 4 concurrent M=32 matmuls via a sliding-window weight tile).
Inputs are packed into 3 DMA transfers; both outputs into 1.
"""

import numpy as np
import ml_dtypes

N, H, NCORES = 1024, 128, 8
BLK = N // NCORES  # 128
P = 128
SCALE = float(1.0 / np.sqrt(np.float32(H)))

CONFIG = dict(
    act_groups=(3,),     # relu col-groups produced on ScalarE
    act_extra_q=0,       # ACT also takes group 2 for q < this
    act_bf16=True,       # ACT relu reads the bf16 negated copy (drops uT_s)
    bench_half_mm=False,  # bench-only: half the relu matmuls (wrong numerics)
    bench_one_relu=False,  # bench-only: one relu tile per quad (wrong numerics)
    bench_dve_1op=False,  # bench-only: single-op DVE relu (wrong numerics)
    fused_transpose=True,  # single 3D dma transpose
    rp_bufs=16,          # relu tile pool depth
    gp_q2=0,             # GPSIMD takes j==2 tiles with q >= 32-gp_q2
    no_memset=True,      # rely on start=True at q==0 instead of PSUM memset
)

# packB (bf16, [128, PB_COLS]) column offsets
PB_HT = 0             # hT [H, N]
PB_HTB = 1024         # hTb [H, BLK]
PB_WQT = 1152         # Wq.T [H, H]
PB_WKT = 1280
PB_WVT = 1408
PB_ZW = 1536          # Zw [H, 63] (w2c at col 31, zeros around)
PB_XO = 1600          # xo rearranged [P, 8, 3] -> cols 3n+c
PB_XV = 1624          # x bf16 rearranged [P, 8, 2] -> cols 2n+c
PB_COLS = 1640

# packF (f32, [128, PF_COLS]) column offsets
PF_HB = 0             # h block + bv [BLK, H]
PF_XB = 128           # x block [BLK, 2]
PF_BQS = 130          # SCALE * bq [H,1]
PF_BK = 131
PF_BE1 = 132
PF_BE1N = 133
PF_B2C = 134
PF_COLS = 135

# packX (f32, [2, PX_COLS]) column offsets
PX_XT = 0             # x.T [2, N]
PX_XTB = 1024         # x_block.T [2, BLK]
PX_WE1T = 1152        # We1.T [2, H]
PX_COLS = 1280

_CACHE = {}


def _build_program(loop_reps=1, **overrides):
    cfg = dict(CONFIG)
    cfg.update(overrides)

    import concourse.tile as tile
    from concourse import bacc, mybir
    from concourse.bass import ts

    f32 = mybir.dt.float32
    bf16 = mybir.dt.bfloat16
    AF = mybir.ActivationFunctionType
    OP = mybir.AluOpType
    AX = mybir.AxisListType

    nc = bacc.Bacc("TRN2")

    packB = nc.dram_tensor("packB", [H, PB_COLS], bf16, kind="ExternalInput")
    packF = nc.dram_tensor("packF", [BLK, PF_COLS], f32, kind="ExternalInput")
    packX = nc.dram_tensor("packX", [2, PX_COLS], f32, kind="ExternalInput")
    out = nc.dram_tensor("out", [BLK, H + 2], f32, kind="ExternalOutput")

    with tile.TileContext(nc) as tc:
        with (
            tc.tile_pool(name="dp", bufs=1) as dp,
            tc.tile_pool(name="rp", bufs=cfg["rp_bufs"]) as rp,
            tc.tile_pool(name="ps_big", bufs=1, space="PSUM") as ps_big,
            tc.tile_pool(name="ps_cw", bufs=1, space="PSUM") as ps_cw,
            tc.tile_pool(name="ps_sm", bufs=2, space="PSUM") as ps_sm,
            tc.tile_pool(name="ps_acc", bufs=1, space="PSUM") as ps_acc,
            tc.tile_pool(name="ps_dx", bufs=1, space="PSUM") as ps_dx,
        ):
            def body():
                # ---------------- packed SBUF loads ----------------
                pB = dp.tile([H, PB_COLS], bf16, tag="pB")
                pF = dp.tile([BLK, PF_COLS], f32, tag="pF")
                pX = dp.tile([2, PX_COLS], f32, tag="pX")
                nc.sync.dma_start(pB[:], packB[:])
                nc.sync.dma_start(pF[:], packF[:])
                nc.sync.dma_start(pX[:], packX[:])

                hT_s = pB[:, PB_HT : PB_HT + N]
                hTb_s = pB[:, PB_HTB : PB_HTB + BLK]
                WqT_s = pB[:, PB_WQT : PB_WQT + H]
                WkT_s = pB[:, PB_WKT : PB_WKT + H]
                WvT_s = pB[:, PB_WVT : PB_WVT + H]
                Z_s = pB[:, PB_ZW : PB_ZW + 63]
                xo_s = pB[:, PB_XO : PB_XO + 24].rearrange("p (n c) -> p n c", n=8)
                xv_s = pB[:, PB_XV : PB_XV + 16].rearrange("p (n c) -> p n c", n=8)
                hb_s = pF[:, PF_HB : PF_HB + H]
                xb_s = pF[:, PF_XB : PF_XB + 2]
                bqs_s = pF[:, PF_BQS : PF_BQS + 1]
                bk_s = pF[:, PF_BK : PF_BK + 1]
                be1_s = pF[:, PF_BE1 : PF_BE1 + 1]
                be1n_s = pF[:, PF_BE1N : PF_BE1N + 1]
                b2c_s = pF[:, PF_B2C : PF_B2C + 1]
                xT_s = pX[:, PX_XT : PX_XT + N]
                xTb_s = pX[:, PX_XTB : PX_XTB + BLK]
                We1T_s = pX[:, PX_WE1T : PX_WE1T + H]

                # [v0 | x] bf16, chunked by j (v0 filled below from psum)
                vx_s = dp.tile([P, 8, H + 2], bf16, tag="vx_s")
                nc.vector.tensor_copy(vx_s[:, :, H : H + 2], xv_s[:])

                # ---------------- u^T and per-i biases ----------------
                uT_ps = ps_big.tile([H, N], f32, tag="big")
                for half in range(2):
                    nc.tensor.matmul(
                        out=uT_ps[:, ts(half, 512)],
                        lhsT=We1T_s,
                        rhs=xT_s[:, ts(half, 512)],
                        start=True,
                        stop=True,
                    )
                uTn_b = dp.tile([H, N], bf16, tag="uTn_b")
                nc.scalar.mul(uTn_b[:], uT_ps[:], -1.0)
                if not cfg["act_bf16"]:
                    uT_s = dp.tile([H, N], f32, tag="uT_s")
                    nc.scalar.copy(uT_s[:], uT_ps[:])

                uTb_ps = ps_sm.tile([H, BLK], f32, tag="sm")
                nc.tensor.matmul(
                    out=uTb_ps[:], lhsT=We1T_s, rhs=xTb_s, start=True, stop=True
                )
                B_s = dp.tile([H, BLK], f32, tag="B_s")
                Bn_s = dp.tile([H, BLK], f32, tag="Bn_s")
                nc.scalar.activation(
                    out=B_s[:], in_=uTb_ps[:], func=AF.Identity,
                    bias=be1_s, scale=1.0,
                )
                nc.scalar.activation(
                    out=Bn_s[:], in_=uTb_ps[:], func=AF.Identity,
                    bias=be1n_s, scale=-1.0,
                )

                # ---------------- q, k, v ----------------
                qT_ps = ps_sm.tile([H, BLK], f32, tag="sm")
                nc.tensor.matmul(
                    out=qT_ps[:], lhsT=WqT_s, rhs=hTb_s, start=True, stop=True
                )
                qT_s = dp.tile([H, BLK], bf16, tag="qT_s")
                nc.scalar.activation(
                    out=qT_s[:], in_=qT_ps[:], func=AF.Identity,
                    bias=bqs_s, scale=SCALE,
                )

                kT_ps = ps_big.tile([H, N], f32, tag="big")
                for half in range(2):
                    nc.tensor.matmul(
                        out=kT_ps[:, ts(half, 512)],
                        lhsT=WkT_s,
                        rhs=hT_s[:, ts(half, 512)],
                        start=True,
                        stop=True,
                    )
                kT_s = dp.tile([H, N], bf16, tag="kT_s")
                nc.scalar.activation(
                    out=kT_s[:], in_=kT_ps[:], func=AF.Identity,
                    bias=bk_s, scale=1.0,
                )

                for n in range(8):
                    v_ps = ps_sm.tile([P, H], f32, tag="sm")
                    nc.tensor.matmul(
                        out=v_ps[:], lhsT=hT_s[:, ts(n, P)], rhs=WvT_s,
                        start=True, stop=True,
                    )
                    nc.scalar.copy(vx_s[:, n, 0:H], v_ps[:])

                # ---------------- attention logits + softmax ----------------
                lg_ps = ps_big.tile([BLK, N], f32, tag="big")
                for half in range(2):
                    nc.tensor.matmul(
                        out=lg_ps[:, ts(half, 512)],
                        lhsT=qT_s[:],
                        rhs=kT_s[:, ts(half, 512)],
                        start=True,
                        stop=True,
                    )
                mx_s = dp.tile([BLK, 1], f32, tag="mx_s")
                nc.vector.tensor_reduce(
                    out=mx_s[:], in_=lg_ps[:], axis=AX.X, op=OP.max, negate=True
                )
                attn_b = dp.tile([BLK, N], bf16, tag="attn_b")
                se_s = dp.tile([BLK, 1], f32, tag="se_s")
                nc.scalar.activation(
                    out=attn_b[:],
                    in_=lg_ps[:],
                    func=AF.Exp,
                    bias=mx_s[:, 0:1],
                    scale=1.0,
                    accum_out=se_s[:, 0:1],
                )
                rs_s = dp.tile([BLK, 1], f32, tag="rs_s")
                nc.vector.reciprocal(rs_s[:], se_s[:])

                # transposed attention chunks (j on partitions) via DMA xbar
                aT_s = dp.tile([P, 8, BLK], bf16, tag="aT_s")
                if cfg["fused_transpose"]:
                    nc.sync.dma_start_transpose(aT_s[:], attn_b[:])
                else:
                    for n in range(8):
                        nc.sync.dma_start_transpose(
                            aT_s[:, n, :], attn_b[:, ts(n, P)]
                        )

                # ---------------- main relu/contract loop ----------------
                cw_ps = ps_cw.tile([BLK, N], f32, tag="cw")
                if not cfg["no_memset"]:
                    # Pre-zero; every matmul then either overwrites
                    # (has_written clear) or accumulates onto zeros — correct
                    # in both HW semantics and CoreSim's pending-zero model.
                    nc.vector.memset(cw_ps[:], 0.0)

                for q in range(32):
                    rts = []
                    for j in range(1 if cfg["bench_one_relu"] else 4):
                        p = 32 * j + q
                        rt = rp.tile([H, N], bf16, tag="relu")
                        on_act = j in cfg["act_groups"] or (
                            j == 2 and q < cfg["act_extra_q"]
                        )
                        on_gp = (not on_act) and (
                            j == 2 and q >= 32 - cfg["gp_q2"]
                        )
                        if on_act:
                            if cfg["act_bf16"]:
                                nc.scalar.activation(
                                    out=rt[:],
                                    in_=uTn_b[:],
                                    func=AF.Relu,
                                    bias=B_s[:, p : p + 1],
                                    scale=1.0,
                                )
                            else:
                                nc.scalar.activation(
                                    out=rt[:],
                                    in_=uT_s[:],
                                    func=AF.Relu,
                                    bias=B_s[:, p : p + 1],
                                    scale=-1.0,
                                )
                        elif cfg["bench_dve_1op"]:
                            eng = nc.gpsimd if on_gp else nc.vector
                            eng.tensor_scalar(
                                out=rt[:],
                                in0=uTn_b[:],
                                scalar1=Bn_s[:, p : p + 1],
                                scalar2=None,
                                op0=OP.max,
                            )
                        else:
                            eng = nc.gpsimd if on_gp else nc.vector
                            eng.tensor_scalar(
                                out=rt[:],
                                in0=uTn_b[:],
                                scalar1=Bn_s[:, p : p + 1],
                                scalar2=0.0,
                                op0=OP.subtract,
                                op1=OP.max,
                            )
                        rts.append(rt)
                    for j in range(4):
                        for half in (0,) if cfg["bench_half_mm"] else (0, 1):
                            nc.tensor.matmul(
                                out=cw_ps[32 * j : 32 * j + 32, ts(half, 512)],
                                lhsT=Z_s[:, 31 - q : 63 - q],
                                rhs=rts[j if not cfg["bench_one_relu"] else 0][
                                    :, ts(half, 512)
                                ],
                                start=(cfg["no_memset"] and q == 0),
                                stop=(q == 31 and j == 3),
                                tile_position=(0, 32 * j),
                                skip_group_check=True,
                            )

                # ---------------- delta_x path ----------------
                # agg matmuls depend only on aT/vx — emit first so PE can run
                # them while the relu loop tail drains.
                agg_ps = ps_acc.tile([BLK, H + 2], f32, tag="agg")
                for n in range(8):
                    nc.tensor.matmul(
                        out=agg_ps[:],
                        lhsT=aT_s[:, n, :],
                        rhs=vx_s[:, n, :],
                        start=(n == 0),
                        stop=(n == 7),
                    )

                # tu -> transpose -> dx in column halves to pipeline the tail
                tu_b = dp.tile([BLK, N], bf16, tag="tu_b")
                tT_s = dp.tile([P, 8, BLK], bf16, tag="tT_s")
                dx_ps = ps_dx.tile([BLK, 3], f32, tag="dx")
                for hh in range(2):
                    cols = ts(hh, 512)
                    nc.vector.tensor_tensor(
                        out=tu_b[:, cols],
                        in0=attn_b[:, cols],
                        in1=cw_ps[:, cols],
                        op=OP.mult,
                    )
                    nc.sync.dma_start_transpose(
                        tT_s[:, 4 * hh : 4 * hh + 4, :], tu_b[:, cols]
                    )
                    for n in range(4 * hh, 4 * hh + 4):
                        nc.tensor.matmul(
                            out=dx_ps[:],
                            lhsT=tT_s[:, n, :],
                            rhs=xo_s[:, n, :],
                            start=(n == 0),
                            stop=(n == 7),
                        )

                # ---------------- final combines (one [BLK, H+2] tile) ------
                o_s = dp.tile([BLK, H + 2], f32, tag="o_s")
                nc.vector.tensor_scalar_mul(
                    o_s[:, 0:H], agg_ps[:, 0:H], rs_s[:, 0:1]
                )
                nc.vector.tensor_tensor(
                    out=o_s[:, 0:H], in0=o_s[:, 0:H], in1=hb_s, op=OP.add
                )

                # A1 = se*b2c + sum_t ; T1 = xb*A1
                A1 = dp.tile([BLK, 1], f32, tag="A1")
                nc.vector.tensor_tensor(
                    out=A1[:], in0=se_s[:], in1=b2c_s, op=OP.mult
                )
                nc.vector.tensor_tensor(
                    out=A1[:], in0=A1[:], in1=dx_ps[:, 2:3], op=OP.add
                )
                T1 = dp.tile([BLK, 2], f32, tag="T1")
                nc.vector.tensor_scalar_mul(T1[:], xb_s, A1[:, 0:1])
                # T2 = b2c*(ae@x) + sum_t_x
                T2 = dp.tile([BLK, 2], f32, tag="T2")
                nc.vector.tensor_scalar_mul(
                    T2[:], agg_ps[:, H : H + 2], b2c_s
                )
                nc.vector.tensor_tensor(
                    out=T2[:], in0=T2[:], in1=dx_ps[:, 0:2], op=OP.add
                )
                nc.vector.tensor_tensor(
                    out=o_s[:, H : H + 2], in0=T1[:], in1=T2[:], op=OP.subtract
                )
                nc.vector.tensor_scalar_mul(
                    o_s[:, H : H + 2], o_s[:, H : H + 2], rs_s[:, 0:1]
                )
                nc.vector.tensor_tensor(
                    out=o_s[:, H : H + 2], in0=o_s[:, H : H + 2], in1=xb_s,
                    op=OP.add,
                )
                nc.sync.dma_start(out[:], o_s[:])

            if loop_reps == 1:
                body()
            else:
                with tc.For_i(
                    0, loop_reps, 1,
                    hint_engines=(
                        nc.tensor.engine, nc.vector.engine, nc.scalar.engine
                    ),
                ):
                    body()

    nc.finalize()
    return nc


def get_program(loop_reps=1, **overrides):
    overrides = {
        k: tuple(v) if isinstance(v, list) else v for k, v in overrides.items()
    }
    key = ("nc", loop_reps, tuple(sorted(overrides.items())))
    if key not in _CACHE:
        _CACHE[key] = _build_program(loop_reps, **overrides)
    return _CACHE[key]


def make_in_maps(inputs):
    bf = ml_dtypes.bfloat16
    f32 = np.float32
    h = np.ascontiguousarray(np.asarray(inputs["h"], f32))
    x = np.ascontiguousarray(np.asarray(inputs["x"], f32))
    Wq = np.asarray(inputs["Wq"], f32)
    bq = np.asarray(inputs["bq"], f32)
    Wk = np.asarray(inputs["Wk"], f32)
    bk = np.asarray(inputs["bk"], f32)
    Wv = np.asarray(inputs["Wv"], f32)
    bv = np.asarray(inputs["bv"], f32)
    We1 = np.asarray(inputs["We1"], f32)
    be1 = np.asarray(inputs["be1"], f32)
    We2 = np.asarray(inputs["We2"], f32)
    be2 = np.asarray(inputs["be2"], f32)
    Wc = np.asarray(inputs["Wc"], f32)
    bc = np.asarray(inputs["bc"], f32)

    w2c = (Wc @ We2).reshape(H)                    # [H]
    b2c_val = float((Wc @ be2)[0] + bc[0])

    # packB (bf16)
    pb = np.zeros((H, PB_COLS), f32)
    pb[:, PB_HT : PB_HT + N] = h.T
    pb[:, PB_WQT : PB_WQT + H] = Wq.T
    pb[:, PB_WKT : PB_WKT + H] = Wk.T
    pb[:, PB_WVT : PB_WVT + H] = Wv.T
    pb[:, PB_ZW + 31] = w2c
    # xo: [x0, x1, 1] chunk-rearranged; xv: x chunk-rearranged
    x3 = np.concatenate([x, np.ones((N, 1), f32)], axis=1)  # [N, 3]
    pb[:, PB_XO : PB_XO + 24] = x3.reshape(8, P, 3).transpose(1, 0, 2).reshape(P, 24)
    pb[:, PB_XV : PB_XV + 16] = x.reshape(8, P, 2).transpose(1, 0, 2).reshape(P, 16)

    # packX (f32)
    px = np.zeros((2, PX_COLS), f32)
    px[:, PX_XT : PX_XT + N] = x.T
    px[:, PX_WE1T : PX_WE1T + H] = We1.T

    in_maps = []
    for c in range(NCORES):
        sl = slice(c * BLK, (c + 1) * BLK)
        pbc = pb.copy()
        pbc[:, PB_HTB : PB_HTB + BLK] = h[sl].T
        pf = np.zeros((BLK, PF_COLS), f32)
        pf[:, PF_HB : PF_HB + H] = h[sl] + bv[None, :]
        pf[:, PF_XB : PF_XB + 2] = x[sl]
        pf[:, PF_BQS] = SCALE * bq
        pf[:, PF_BK] = bk
        pf[:, PF_BE1] = be1
        pf[:, PF_BE1N] = -be1
        pf[:, PF_B2C] = b2c_val
        pxc = px.copy()
        pxc[:, PX_XTB : PX_XTB + BLK] = x[sl].T
        in_maps.append(
            {
                "packB": np.ascontiguousarray(pbc).astype(bf),
                "packF": np.ascontiguousarray(pf),
                "packX": np.ascontiguousarray(pxc),
            }
        )
    return in_maps


def kernel(**inputs):
    from concourse.bass_utils import run_bass_kernel_spmd

    nc = get_program()
    in_maps = make_in_maps(inputs)
    res = run_bass_kernel_spmd(nc, in_maps, core_ids=list(range(NCORES)))
    o = np.concatenate([r["out"] for r in res.results], axis=0)
    return (
        np.ascontiguousarray(o[:, 0:H]).astype(np.float32),
        np.ascontiguousarray(o[:, H : H + 2]).astype(np.float32),
    )
